# revision 8
# baseline (speedup 1.0000x reference)
"""Trainium2 Bass kernel for a pre-norm transformer block with patch-local
(serialized-order) attention.

Strategy: the whole block is row-independent except attention, which mixes
rows only within contiguous 128-row patches of the *serialized* order.  So we
gather feat by `order` on the host, shard the serialized rows across the 8
cores (2048 patches -> 256 patches/core... actually 1024 patches, 128/core),
run the entire block per-core with zero cross-core traffic, and scatter back
on the host.

Per-core dataflow (row-major f32 residual stream, bf16 matmul operands):
  LN1 (bn_stats) -> xn bf16 -> PE-transpose -> xnT
  qkT = Wqk'(stationary) @ xnT        [j, rows] (LN gain + attn scale folded)
  v   = xnT(stationary) @ Wv'         [rows, vcols] row-major
  S^T_h = kT_h(stat) @ qT_h           [j, i] per patch/head, contract d=32
  A^T_h = exp(S^T_h)  (no max-sub: logits are O(1) by construction)
  [O'_h | denom_h] = A^T_h(stat) @ [v_h | ones]   row-major + softmax denom
  attnO = O' * recip(denom)  (per-partition scalar)
  proj: y = attnO^T(stat) @ Wp, x1 = feat + y
  LN2 -> xn2T; h1T = W1'(stat) @ xn2T; gelu; h2 = h1T(stat) @ W2; x2 = x1 + h2
"""

import numpy as np

N, C, H, K, HID = 131072, 256, 8, 128, 1024
D = C // H          # 32
NCORES = 8
R = N // NCORES     # 16384 rows per core
SCALE = D ** -0.5

CPC = 4             # patches per chunk
RW = K * CPC        # 512 rows per chunk

_CACHE = {}


def _build(n_chunks, has_vbias, has_pbias):
    from contextlib import ExitStack
    import concourse.bass as bass
    import concourse.bacc as bacc
    import concourse.tile as tile
    from concourse import mybir
    from concourse.masks import make_identity

    f32 = mybir.dt.float32
    bf16 = mybir.dt.bfloat16
    AF = mybir.ActivationFunctionType
    OP = mybir.AluOpType

    rows_total = RW * n_chunks

    nc = bacc.Bacc()
    feat = nc.dram_tensor("feat", [rows_total, C], f32, kind="ExternalInput")
    wqk = nc.dram_tensor("wqk", [2, 128, 512], bf16, kind="ExternalInput")
    wv = nc.dram_tensor("wv", [2, 128, 256], bf16, kind="ExternalInput")
    wp = nc.dram_tensor("wp", [2, 128, 256], bf16, kind="ExternalInput")
    w1 = nc.dram_tensor("w1", [2, 128, 1024], bf16, kind="ExternalInput")
    w2 = nc.dram_tensor("w2", [8, 128, 256], bf16, kind="ExternalInput")
    bqk = nc.dram_tensor("bqk", [128, 4], f32, kind="ExternalInput")
    b1f = nc.dram_tensor("b1f", [128, 8], f32, kind="ExternalInput")
    if has_vbias:
        bv = nc.dram_tensor("bv", [1, 256], f32, kind="ExternalInput")
    if has_pbias:
        bp = nc.dram_tensor("bp", [1, 256], f32, kind="ExternalInput")
    out = nc.dram_tensor("out", [rows_total, C], f32, kind="ExternalOutput")

    with ExitStack() as ctx:
        tc = ctx.enter_context(tile.TileContext(nc))

        const = ctx.enter_context(tc.tile_pool(name="const", bufs=1))
        # resident weights
        wqk_sb = const.tile([128, 2, 512], bf16)
        wv_sb = const.tile([128, 2, 256], bf16)
        wp_sb = const.tile([128, 2, 256], bf16)
        w1_sb = const.tile([128, 2, 1024], bf16)
        w2_sb = const.tile([128, 8, 256], bf16)
        for cb in range(2):
            nc.sync.dma_start(wqk_sb[:, cb, :], wqk[cb])
            nc.sync.dma_start(wv_sb[:, cb, :], wv[cb])
            nc.sync.dma_start(wp_sb[:, cb, :], wp[cb])
            nc.sync.dma_start(w1_sb[:, cb, :], w1[cb])
        for hb in range(8):
            nc.sync.dma_start(w2_sb[:, hb, :], w2[hb])
        bqk_sb = const.tile([128, 4], f32)
        nc.sync.dma_start(bqk_sb, bqk[:, :])
        b1f_sb = const.tile([128, 8], f32)
        nc.sync.dma_start(b1f_sb, b1f[:, :])
        if has_vbias:
            bv_sb = const.tile([128, 256], f32)
            nc.sync.dma_start(
                bv_sb, bass.AP(tensor=bv.tensor, offset=bv.offset,
                               ap=[[0, 128]] + list(bv.ap[1:])))
        if has_pbias:
            bp_sb = const.tile([128, 256], f32)
            nc.sync.dma_start(
                bp_sb, bass.AP(tensor=bp.tensor, offset=bp.offset,
                               ap=[[0, 128]] + list(bp.ap[1:])))
        ident = const.tile([128, 128], bf16)
        make_identity(nc, ident)
        eps_t = const.tile([128, 1], f32)
        nc.vector.memset(eps_t, 1e-5)

        # pools
        feat_p = ctx.enter_context(tc.tile_pool(name="feat", bufs=10))
        small = ctx.enter_context(tc.tile_pool(name="small", bufs=12))
        xn_p = ctx.enter_context(tc.tile_pool(name="xn", bufs=3))
        xnT_p = ctx.enter_context(tc.tile_pool(name="xnT", bufs=2))
        qkT_p = ctx.enter_context(tc.tile_pool(name="qkT", bufs=2))
        v_p = ctx.enter_context(tc.tile_pool(name="v", bufs=3))
        at_p = ctx.enter_context(tc.tile_pool(name="at", bufs=4))
        ao_p = ctx.enter_context(tc.tile_pool(name="ao", bufs=3))
        x1_p = ctx.enter_context(tc.tile_pool(name="x1", bufs=6))
        h1_p = ctx.enter_context(tc.tile_pool(name="h1", bufs=2))
        out_p = ctx.enter_context(tc.tile_pool(name="xout", bufs=3))

        psBig = ctx.enter_context(tc.tile_pool(name="psBig", bufs=3, space="PSUM"))
        psMed = ctx.enter_context(tc.tile_pool(name="psMed", bufs=2, space="PSUM"))
        psSm = ctx.enter_context(tc.tile_pool(name="psSm", bufs=3, space="PSUM"))

        def layernorm_to_T(src_tiles, dst_T, pidx_base):
            """src row-major f32 [128,256] tiles -> normalized bf16 transposed
            dst_T [128, 2, RW] (c-blk, rows)."""
            for p, ft in enumerate(src_tiles):
                stats = small.tile([128, 6], f32, tag="stats")
                nc.vector.bn_stats(stats, ft)
                mv = small.tile([128, 2], f32, tag="mv")
                nc.vector.bn_aggr(mv, stats)
                rs = small.tile([128, 1], f32, tag="rs")
                nc.scalar.activation(rs, mv[:, 1:2], AF.Sqrt, bias=eps_t, scale=1.0)
                nc.vector.reciprocal(rs, rs)
                xn = xn_p.tile([128, 256], bf16, tag="xn")
                # (ft - mean) * rstd via two per-partition scalar operands
                nc.vector.tensor_scalar(
                    out=xn, in0=ft, scalar1=mv[:, 0:1], scalar2=rs,
                    op0=OP.subtract, op1=OP.mult)
                for cb in range(2):
                    tp = psSm.tile([128, 128], bf16, tag="small")
                    nc.tensor.transpose(tp, xn[:, 128 * cb:128 * (cb + 1)], ident)
                    nc.vector.tensor_copy(
                        dst_T[:, cb, 128 * p:128 * (p + 1)], tp)

        for ci in range(n_chunks):
            r0 = ci * RW

            # ---- load + LN1 ----
            fts = []
            for p in range(CPC):
                ft = feat_p.tile([128, 256], f32, tag="feat")
                nc.sync.dma_start(ft, feat[r0 + 128 * p: r0 + 128 * (p + 1), :])
                fts.append(ft)
            xnT = xnT_p.tile([128, 2, RW], bf16, tag="xnT")
            layernorm_to_T(fts, xnT, 0)

            # ---- qkT: [j(4 blk x 128), rows] ----
            qkT = qkT_p.tile([128, 4, RW], bf16, tag="qkT")
            for jb in range(4):
                ps = psBig.tile([128, RW], f32, tag="big")
                for cb in range(2):
                    nc.tensor.matmul(
                        ps, lhsT=wqk_sb[:, cb, 128 * jb:128 * (jb + 1)],
                        rhs=xnT[:, cb, :], start=(cb == 0), stop=(cb == 1))
                nc.scalar.activation(
                    qkT[:, jb, :], ps, AF.Identity,
                    bias=bqk_sb[:, jb:jb + 1], scale=1.0)

            # ---- v row-major per patch: [128 rows, 8, 33] (ones col for denom) ----
            vs = []
            for p in range(CPC):
                vt = v_p.tile([128, 8, 33], bf16, tag="v")
                nc.vector.memset(vt[:, :, 32:33], 1.0)
                ps = psMed.tile([128, 256], f32, tag="med")
                if has_vbias:
                    nc.vector.tensor_copy(ps, bv_sb)
                for cb in range(2):
                    nc.tensor.matmul(
                        ps, lhsT=xnT[:, cb, 128 * p:128 * (p + 1)],
                        rhs=wv_sb[:, cb, :],
                        start=(cb == 0 and not has_vbias), stop=(cb == 1))
                nc.vector.tensor_copy(
                    vt[:, :, 0:32], ps.rearrange("p (h d) -> p h d", h=8))
                vs.append(vt)

            # ---- attention + proj + residual per patch ----
            x1s = []
            for p in range(CPC):
                rsl = slice(128 * p, 128 * (p + 1))
                attnO = ao_p.tile([128, 256], bf16, tag="ao")
                for g in range(2):          # 4-head groups
                    pav = psSm.tile([128, 132], f32, tag="small")
                    ats = []
                    for hh in range(4):
                        h = 4 * g + hh
                        qb, hp = divmod(h, 4)
                        po = slice(32 * hp, 32 * (hp + 1))
                        ps_s = psSm.tile([128, 128], f32, tag="small")
                        nc.tensor.matmul(
                            ps_s, lhsT=qkT[po, 2 + qb, rsl],
                            rhs=qkT[po, qb, rsl], start=True, stop=True,
                            tile_position=(32 * hp, 0))
                        at = at_p.tile([128, 128], bf16, tag="at")
                        nc.scalar.activation(at, ps_s, AF.Exp)
                        ats.append(at)
                    for hh in range(4):
                        h = 4 * g + hh
                        nc.tensor.matmul(
                            pav[:, 33 * hh:33 * hh + 33], lhsT=ats[hh],
                            rhs=vs[p][:, h, :], start=True, stop=True)
                    rc = small.tile([128, 4], f32, tag="rc")
                    nc.vector.reciprocal(
                        rc, pav.rearrange("p (h e) -> p h e", h=4)[:, :, 32])
                    for hh in range(4):
                        h = 4 * g + hh
                        dst = attnO[:, 32 * h:32 * (h + 1)]
                        src = pav[:, 33 * hh:33 * hh + 32]
                        if hh % 2 == 0:
                            nc.vector.tensor_scalar_mul(dst, src, rc[:, hh:hh + 1])
                        else:
                            nc.scalar.activation(
                                dst, src, AF.Copy, scale=rc[:, hh:hh + 1])
                # transpose attnO -> OT
                oT = xn_p.tile([128, 2, 128], bf16, tag="oT")
                for cb in range(2):
                    tp = psSm.tile([128, 128], bf16, tag="small")
                    nc.tensor.transpose(tp, attnO[:, 128 * cb:128 * (cb + 1)], ident)
                    nc.scalar.copy(oT[:, cb, :], tp)
                # proj + residual
                psy = psMed.tile([128, 256], f32, tag="med")
                if has_pbias:
                    nc.vector.tensor_copy(psy, bp_sb)
                for cb in range(2):
                    nc.tensor.matmul(
                        psy, lhsT=oT[:, cb, :], rhs=wp_sb[:, cb, :],
                        start=(cb == 0 and not has_pbias), stop=(cb == 1))
                x1 = x1_p.tile([128, 256], f32, tag="x1")
                nc.vector.tensor_add(x1, fts[p], psy)
                x1s.append(x1)

            # ---- LN2 -> xn2T ----
            xn2T = xnT_p.tile([128, 2, RW], bf16, tag="xn2T")
            layernorm_to_T(x1s, xn2T, 0)

            # ---- MLP ----
            h1g = h1_p.tile([128, 8, RW], bf16, tag="h1")
            for jb in range(8):
                ps = psBig.tile([128, RW], f32, tag="big")
                for cb in range(2):
                    nc.tensor.matmul(
                        ps, lhsT=w1_sb[:, cb, 128 * jb:128 * (jb + 1)],
                        rhs=xn2T[:, cb, :], start=(cb == 0), stop=(cb == 1))
                nc.scalar.activation(
                    h1g[:, jb, :], ps, AF.Gelu_apprx_tanh,
                    bias=b1f_sb[:, jb:jb + 1], scale=1.0)
            for p in range(CPC):
                ps2 = psMed.tile([128, 256], f32, tag="med")
                for hb in range(8):
                    nc.tensor.matmul(
                        ps2, lhsT=h1g[:, hb, 128 * p:128 * (p + 1)],
                        rhs=w2_sb[:, hb, :], start=(hb == 0), stop=(hb == 7))
                x2 = out_p.tile([128, 256], f32, tag="x2")
                nc.vector.tensor_add(x2, x1s[p], ps2)
                nc.sync.dma_start(
                    out[r0 + 128 * p: r0 + 128 * (p + 1), :], x2)

    nc.finalize()
    return nc


def _prep_weights(ln1_g, ln1_b, w_qkv, b_qkv, w_proj, b_proj,
                  ln2_g, ln2_b, w1, b1, w2, b2):
    import ml_dtypes
    bf = ml_dtypes.bfloat16
    wq = (ln1_g[:, None] * w_qkv).astype(np.float32).copy()
    bq = (ln1_b @ w_qkv + b_qkv).astype(np.float32).copy()
    wq[:, :256] *= SCALE
    bq[:256] *= SCALE
    w1f = (ln2_g[:, None] * w1).astype(np.float32)
    b1f = (ln2_b @ w1 + b1).astype(np.float32)
    m = {
        "wqk": np.ascontiguousarray(wq[:, :512].reshape(2, 128, 512)).astype(bf),
        "wv": np.ascontiguousarray(wq[:, 512:768].reshape(2, 128, 256)).astype(bf),
        "wp": np.ascontiguousarray(w_proj.reshape(2, 128, 256)).astype(bf),
        "w1": np.ascontiguousarray(w1f.reshape(2, 128, 1024)).astype(bf),
        "w2": np.ascontiguousarray(w2.reshape(8, 128, 256)).astype(bf),
        "bqk": np.ascontiguousarray(bq[:512].reshape(4, 128).T).astype(np.float32),
        "b1f": np.ascontiguousarray(b1f.reshape(8, 128).T).astype(np.float32),
    }
    has_vbias = bool(np.any(bq[512:768]))
    has_pbias = bool(np.any(b_proj))
    if has_vbias:
        m["bv"] = bq[512:768].reshape(1, 256).astype(np.float32)
    if has_pbias:
        m["bp"] = np.asarray(b_proj, np.float32).reshape(1, 256)
    return m, has_vbias, has_pbias


PROFILE = False
LAST_EXEC_NS = None


def kernel(feat, ln1_g, ln1_b, w_qkv, b_qkv, w_proj, b_proj,
           ln2_g, ln2_b, w1, b1, w2, b2, order, inverse):
    global LAST_EXEC_NS
    import sys
    if "/opt/trn_rl_repo" not in sys.path:
        sys.path.insert(0, "/opt/trn_rl_repo")
    from concourse.bass_utils import run_bass_kernel_spmd

    feat = np.asarray(feat, np.float32)
    order_np = np.asarray(order)
    args = [np.asarray(a, np.float32) for a in
            (ln1_g, ln1_b, w_qkv, b_qkv, w_proj, b_proj,
             ln2_g, ln2_b, w1, b1, w2, b2)]
    wmap, has_vbias, has_pbias = _prep_weights(*args)
    b2_np = args[11]

    n_chunks = R // RW
    key = (n_chunks, has_vbias, has_pbias)
    if key not in _CACHE:
        _CACHE[key] = _build(*key)
    nc = _CACHE[key]

    feat_g = feat[order_np]          # serialized order
    in_maps = []
    for m in range(NCORES):
        im = dict(wmap)
        im["feat"] = feat_g[m * R:(m + 1) * R]
        in_maps.append(im)

    res = run_bass_kernel_spmd(nc, in_maps, core_ids=list(range(NCORES)),
                               trace=PROFILE)
    if PROFILE:
        LAST_EXEC_NS = res.exec_time_ns
    out_serial = np.concatenate([res.results[m]["out"] for m in range(NCORES)],
                                axis=0)
    out_serial = out_serial + b2_np[None, :]
    final = np.empty((N, C), np.float32)
    final[order_np] = out_serial
    return final
